# revision 29
# baseline (speedup 1.0000x reference)
"""Trainium2 Bass kernel for AdaBlock: binarized 3x3 conv (256->128) + bias +
PReLU + bias + scaled shortcut + pixel_unshuffle(2).

Strategy: data-parallel across 8 NeuronCores (2 images each). The conv is an
implicit GEMM in fp8 with DoubleRow matmuls: weights are the conv signs
(+/-1, exact in fp8e4), the per-channel scale alpha = mean|w| is folded into
the epilogue activation scales. Activations are quantized to fp8e4 on host
(x_hat) plus an fp8e4 residual (e_hat = fp8(x - x_hat)). Each 512-pixel PSUM
tile (4 rows x 128 cols) accumulates 13 DoubleRow matmuls (K=256 = both
input-channel chunks per instruction, at 2 MACs/cell/cycle): 9 taps on x_hat
+ NCORR taps re-applied to e_hat, which cancels enough quantization noise to
keep rel err ~1.6e-2 (fp8-only would be 2.1e-2). Tensor-engine work is
13/18 of the bf16 baseline. Epilogue per tile, with c = sign-conv PSUM,
v = alpha*c + b1, na = -prelu_a <= 0:
  r1 = relu(alpha*c + b1), r2 = relu(na*alpha*c + na*b1)   (2 ScalarE acts)
  out = r1 - r2 + (b2 + scale * x[:128])                   (2 VectorE ops)
The shortcut path reads a separate bf16 copy of x[:, :128]. Halo rows/cols
are zero-padded in SBUF (width 130); first/last blocks are shrunk and dummy
warm-up matmuls run while the first DMA lands; the Tile exit barrier is
trimmed. pixel_unshuffle is a host-side reshape after gathering.
"""

import numpy as np
import ml_dtypes

B_FULL = 16
B_CORE = 2          # images per core (16 / 8 cores)
CIN = 256
COUT = 128
H = W = 128
RB = 16             # output rows per block
NBLK = H // RB      # 8 blocks per image
WP = W + 2          # padded width in SBUF
N_CORES = 8
NCORR = 2           # taps whose residual is corrected with e_hat (kh=0 row)

LAST_EXEC_NS = None
LAST_PROFILE = None

_cache = {}


def _build():
    import concourse.mybir as mybir
    import concourse.tile as tile
    from concourse import bacc

    fp32 = mybir.dt.float32
    bf16 = mybir.dt.bfloat16
    fp8 = mybir.dt.float8e4

    nc = bacc.Bacc("TRN2", target_bir_lowering=False, debug=False,
                   num_devices=N_CORES)

    class FastExitTileContext(tile.TileContext):
        # Tile's exit emits drain + barrier + sem-clears + barrier (~9-17us).
        # Keep only the drain (output DMAs must land); the engine barriers
        # and HW sem clears are dropped — the runtime resets semaphores at
        # NEFF load and waits for all engine programs itself.
        def _drain_and_barrier(self, tick_clock, wait_clock):
            drain_inst = self.nc.sync.drain()
            wait_clock.add_sem_waits(
                drain_inst.ins,
                tile.ScopedClock({None: tick_clock.global_clock}))
            popped = self.nc._tile_sem_poison_stack.pop()
            assert popped is self._sem_poison
            from concourse.bass import SemaphoreHandle
            sem_nums = [s.num if isinstance(s, SemaphoreHandle) else s
                        for s in self.sems.allocated().values()]
            self.nc._state.prepend_free_semaphores(sem_nums)

    # width-padded on host (WP=130, zero left/right cols), partition-major so
    # each block DMA moves rows*130B contiguous runs per (partition, chunk)
    x_ext = nc.dram_tensor("x8", [B_CORE, 128, 2, H, WP], fp8,
                           kind="ExternalInput")
    e_ext = nc.dram_tensor("e8", [B_CORE, 128, 2, H, WP], fp8,
                           kind="ExternalInput")
    s_ext = nc.dram_tensor("xs", [B_CORE, COUT, H, W], bf16,
                           kind="ExternalInput")
    w_ext = nc.dram_tensor("w8", [128, 2, 3, 3, COUT], fp8,
                           kind="ExternalInput")
    # packed per-channel params: cols = (b1, alpha, prelu_a, b2, s)
    p_ext = nc.dram_tensor("p", [COUT, 5], fp32, kind="ExternalInput")
    out_ext = nc.dram_tensor("out", [B_CORE, COUT, H, W], bf16,
                             kind="ExternalOutput")

    AF = mybir.ActivationFunctionType
    OP = mybir.AluOpType
    DR = mybir.MatmulPerfMode.DoubleRow

    TAPS = [(kh, kw) for kh in range(3) for kw in range(3)]
    # main taps on x_hat, then NCORR correction taps on e_hat
    MM = [(kh, kw, 0) for kh, kw in TAPS] + \
         [(kh, kw, 1) for kh, kw in TAPS[:NCORR]]

    with FastExitTileContext(nc) as tc:
        with tc.tile_pool(name="const", bufs=1) as cpool, \
             tc.tile_pool(name="xin", bufs=6) as xpool, \
             tc.tile_pool(name="outp", bufs=3) as opool, \
             tc.tile_pool(name="eps", bufs=4) as epool, \
             tc.tile_pool(name="psum", bufs=2, space="PSUM") as pspool:

            wts = {kh: cpool.tile([128, 2, 3, COUT], fp8,
                                  name=f"wt{kh}", tag=f"wt{kh}")
                   for kh in range(3)}
            pt = cpool.tile([COUT, 5], fp32)
            b1 = pt[:, 0:1]
            al = pt[:, 1:2]
            pa = pt[:, 2:3]
            b2 = pt[:, 3:4]
            sv = pt[:, 4:5]

            def load_x(b, r0, nrows, ext, tg, halo=None):
                # halo defaults to nrows+2 (rows r0-1 .. r0+nrows); the e_hat
                # tile only needs rows r0-1 .. r0+nrows-2 (kh=0 taps), so its
                # halo is nrows
                halo = nrows + 2 if halo is None else halo
                xb = xpool.tile([128, 2, halo, WP], fp8, tag=tg, name=tg)
                lo = max(r0 - 1, 0)
                hi = min(r0 - 1 + halo, H)
                off = lo - (r0 - 1)
                if off:
                    nc.vector.memset(xb[:, :, 0, :], 0.0)
                if hi - lo + off < halo:
                    nc.vector.memset(xb[:, :, halo - 1, :], 0.0)
                nc.sync.dma_start(
                    xb[:, :, off:off + (hi - lo), :],
                    ext[b, :, :, lo:hi, :])
                return xb

            def emit_block(b, r0, nrows, gsplit, xbs=None):
                if xbs is None:
                    xbs = (load_x(b, r0, nrows, x_ext, "xb"),
                           load_x(b, r0, nrows, e_ext, "eb", halo=nrows))
                xb, eb = xbs

                sb = xpool.tile([COUT, nrows, W], bf16, tag="sb", name="sb")
                nc.sync.dma_start(sb[:], s_ext[b, :, r0:r0 + nrows, :])
                ob = opool.tile([COUT, nrows, W], bf16, tag="ob", name="ob")
                sc = opool.tile([COUT, nrows, W], fp32, tag="sc", name="sc")
                # shortcut = scale * x[:, :128] + b2 (VectorE: ScalarE is the
                # busier engine in the epilogue)
                nc.vector.tensor_scalar(sc[:], sb[:], sv, b2,
                                        OP.mult, OP.add)

                NG = nrows // 4

                def epilogue(g, ps):
                    # prelu(alpha*c + b1) in one ACT op (per-channel slope),
                    # then add the precomputed shortcut
                    r1 = epool.tile([COUT, 512], fp32, tag="r1", name="r1")
                    nc.scalar.activation(r1[:], ps[:], AF.Prelu,
                                         bias=b1, scale=al, alpha=pa)
                    nc.vector.tensor_tensor(
                        ob[:, 4 * g:4 * g + 4, :], r1[:],
                        sc[:, 4 * g:4 * g + 4, :], OP.add)

                for gs in range(0, NG, gsplit):
                    gset = list(range(gs, min(gs + gsplit, NG)))
                    pss = {}
                    for idx, (kh, kw, isc) in enumerate(MM):
                        src = eb if isc else xb
                        for g in gset:
                            if idx == 0:
                                pss[g] = pspool.tile(
                                    [COUT, 512], fp32,
                                    tag=f"ps{g}", name=f"ps{g}")
                            nc.tensor.matmul(
                                pss[g][:],
                                wts[kh][:, :, kw, :],
                                src[:, :, 4 * g + kh:4 * g + kh + 4,
                                    kw:kw + W],
                                start=(idx == 0), stop=(idx == len(MM) - 1),
                                perf_mode=DR)
                    for g in gset:
                        epilogue(g, pss[g])

                nc.sync.dma_start(out_ext[b, :, r0:r0 + nrows, :], ob[:])

            # graduated ramp-up at the start (small DMAs first so PE starts
            # early), steady 16-row blocks, 4-row tail for a short epilogue
            blocks = [(0, 0, 4), (0, 4, 8), (0, 12, 8), (0, 20, 12)]
            r = 32
            while r < H:
                blocks.append((0, r, RB))
                r += RB
            for blk in range(NBLK):
                blocks.append((1, blk * RB, RB))
            blocks[-1] = (B_CORE - 1, (NBLK - 1) * RB, RB - 4)
            blocks.append((B_CORE - 1, H - 4, 4))

            # PE warm-up: dummy matmuls on memset data fill the idle window
            # while the first input DMA lands, releasing the HAM throttle
            dmy = cpool.tile([128, 640], mybir.dt.bfloat16)
            nc.vector.memset(dmy[:, 0:2], 0.0)
            dps = pspool.tile([COUT, 512], fp32, tag="ps0", name="dps")
            for _ in range(20):
                nc.tensor.matmul(dps[:, :256], dmy[:, :128],
                                 dmy[:, 128:384], start=True, stop=True)

            # first block's x before the big const DMAs so PE starts early
            xb_first = load_x(0, 0, 4, x_ext, "xb")
            nc.sync.dma_start(wts[0][:], w_ext[:, :, 0])
            nc.sync.dma_start(wts[1][:], w_ext[:, :, 1])
            eb_first = load_x(0, 0, 4, e_ext, "eb", halo=4)
            nc.sync.dma_start(wts[2][:], w_ext[:, :, 2])
            nc.sync.dma_start(pt[:], p_ext[:])

            for i, (b, r0, nrows) in enumerate(blocks):
                last = i == len(blocks) - 1
                emit_block(b, r0, nrows,
                           min(2, nrows // 4) if last else nrows // 4,
                           xbs=(xb_first, eb_first) if i == 0 else None)

    nc.compile()
    return nc


def kernel(x, conv_w, move1_b, prelu_w, move2_b, scale, _trace=False):
    global LAST_EXEC_NS, LAST_PROFILE
    x = np.asarray(x)
    conv_w = np.asarray(conv_w)
    move1_b = np.asarray(move1_b)
    prelu_w = np.asarray(prelu_w)
    move2_b = np.asarray(move2_b)
    scale = np.asarray(scale)
    assert x.shape == (B_FULL, CIN, H, W), x.shape

    fp8 = ml_dtypes.float8_e4m3

    # --- host-side weight binarization (exact reference math, fp32) ---
    w32 = conv_w.astype(np.float32)
    alpha = np.mean(np.abs(w32), axis=(1, 2, 3))                  # [O]
    # device weights are the signs; alpha is folded into the epilogue
    wl = np.sign(w32).reshape(COUT, 2, 128, 3, 3).transpose(2, 1, 3, 4, 0)
    wl = np.ascontiguousarray(wl).astype(fp8)

    al32 = prelu_w.astype(np.float32)
    b132 = move1_b.astype(np.float32)
    params = np.stack([
        b132,
        alpha,
        al32,
        move2_b.astype(np.float32),
        np.full((COUT,), float(scale[0]), np.float32),
    ], axis=1)
    params = np.ascontiguousarray(params)

    x32 = x.astype(np.float32)
    xq = x32.astype(fp8)                                  # x_hat
    eq = (x32 - xq.astype(np.float32)).astype(fp8)        # e_hat
    # width-pad to WP=130 and lay out [B, 128part, 2chunk, H, WP]
    def _pad(a):
        p = np.zeros((B_FULL, 2, 128, H, WP), fp8)
        p[:, :, :, :, 1:1 + W] = a.reshape(B_FULL, 2, 128, H, W)
        return np.ascontiguousarray(p.transpose(0, 2, 1, 3, 4))
    xq = _pad(xq)
    eq = _pad(eq)
    xs = x32[:, :COUT].astype(ml_dtypes.bfloat16)

    if "nc" not in _cache:
        _cache["nc"] = _build()
    nc = _cache["nc"]

    in_maps = []
    for i in range(N_CORES):
        sl = slice(i * B_CORE, (i + 1) * B_CORE)
        in_maps.append({
            "x8": np.ascontiguousarray(xq[sl]),
            "e8": np.ascontiguousarray(eq[sl]),
            "xs": np.ascontiguousarray(xs[sl]),
            "w8": wl,
            "p": params,
        })

    from concourse.bass_utils import run_bass_kernel_spmd
    res = run_bass_kernel_spmd(nc, in_maps, core_ids=list(range(N_CORES)),
                               trace=_trace)
    LAST_EXEC_NS = res.exec_time_ns
    LAST_PROFILE = res
    out = np.concatenate([res.results[i]["out"] for i in range(N_CORES)],
                         axis=0).astype(np.float32)   # [16,128,128,128]

    # pixel_unshuffle2: [B,C,H,W] -> [B,C*4,H/2,W/2]
    B, C, HH, WW = out.shape
    out = out.reshape(B, C, HH // 2, 2, WW // 2, 2)
    out = out.transpose(0, 1, 3, 5, 2, 4)
    return np.ascontiguousarray(out.reshape(B, C * 4, HH // 2, WW // 2))


# revision 30
# speedup vs baseline: 1.0145x; 1.0145x over previous
"""Trainium2 Bass kernel for AdaBlock: binarized 3x3 conv (256->128) + bias +
PReLU + bias + scaled shortcut + pixel_unshuffle(2).

Strategy: data-parallel across 8 NeuronCores (2 images each). The conv is an
implicit GEMM in fp8 with DoubleRow matmuls: weights are the conv signs
(+/-1, exact in fp8e4), the per-channel scale alpha = mean|w| is folded into
the epilogue activation scales. Activations are quantized to fp8e4 on host
(x_hat) plus an fp8e4 residual (e_hat = fp8(x - x_hat)). Each 512-pixel PSUM
tile (4 rows x 128 cols) accumulates 13 DoubleRow matmuls (K=256 = both
input-channel chunks per instruction, at 2 MACs/cell/cycle): 9 taps on x_hat
+ NCORR taps re-applied to e_hat, which cancels enough quantization noise to
keep rel err ~1.6e-2 (fp8-only would be 2.1e-2). Tensor-engine work is
13/18 of the bf16 baseline. Epilogue per tile, with c = sign-conv PSUM,
v = alpha*c + b1, na = -prelu_a <= 0:
  r1 = relu(alpha*c + b1), r2 = relu(na*alpha*c + na*b1)   (2 ScalarE acts)
  out = r1 - r2 + (b2 + scale * x[:128])                   (2 VectorE ops)
The shortcut path reads a separate bf16 copy of x[:, :128]. Halo rows/cols
are zero-padded in SBUF (width 130); first/last blocks are shrunk and dummy
warm-up matmuls run while the first DMA lands; the Tile exit barrier is
trimmed. pixel_unshuffle is a host-side reshape after gathering.
"""

import numpy as np
import ml_dtypes

B_FULL = 16
B_CORE = 2          # images per core (16 / 8 cores)
CIN = 256
COUT = 128
H = W = 128
RB = 16             # output rows per block
NBLK = H // RB      # 8 blocks per image
WP = W + 2          # padded width in SBUF
N_CORES = 8
NCORR = 2           # taps whose residual is corrected with e_hat (kh=0 row)

LAST_EXEC_NS = None
LAST_PROFILE = None

_cache = {}


def _build():
    import concourse.mybir as mybir
    import concourse.tile as tile
    from concourse import bacc

    fp32 = mybir.dt.float32
    bf16 = mybir.dt.bfloat16
    fp8 = mybir.dt.float8e4

    nc = bacc.Bacc("TRN2", target_bir_lowering=False, debug=False,
                   num_devices=N_CORES)

    class FastExitTileContext(tile.TileContext):
        # Tile's exit emits drain + barrier + sem-clears + barrier (~9-17us).
        # Keep only the drain (output DMAs must land); the engine barriers
        # and HW sem clears are dropped — the runtime resets semaphores at
        # NEFF load and waits for all engine programs itself.
        def _drain_and_barrier(self, tick_clock, wait_clock):
            drain_inst = self.nc.sync.drain()
            wait_clock.add_sem_waits(
                drain_inst.ins,
                tile.ScopedClock({None: tick_clock.global_clock}))
            popped = self.nc._tile_sem_poison_stack.pop()
            assert popped is self._sem_poison
            from concourse.bass import SemaphoreHandle
            sem_nums = [s.num if isinstance(s, SemaphoreHandle) else s
                        for s in self.sems.allocated().values()]
            self.nc._state.prepend_free_semaphores(sem_nums)

    # width-padded on host (WP=130, zero left/right cols), partition-major so
    # each block DMA moves rows*130B contiguous runs per (partition, chunk)
    x_ext = nc.dram_tensor("x8", [B_CORE, 128, 2, H, WP], fp8,
                           kind="ExternalInput")
    e_ext = nc.dram_tensor("e8", [B_CORE, 128, 2, H, WP], fp8,
                           kind="ExternalInput")
    s_ext = nc.dram_tensor("xs", [B_CORE, COUT, H, W], bf16,
                           kind="ExternalInput")
    w_ext = nc.dram_tensor("w8", [128, 2, 3, 3, COUT], fp8,
                           kind="ExternalInput")
    # packed per-channel params: cols = (b1, alpha, prelu_a, b2, s)
    p_ext = nc.dram_tensor("p", [COUT, 5], fp32, kind="ExternalInput")
    out_ext = nc.dram_tensor("out", [B_CORE, COUT, H, W], bf16,
                             kind="ExternalOutput")

    AF = mybir.ActivationFunctionType
    OP = mybir.AluOpType
    DR = mybir.MatmulPerfMode.DoubleRow

    TAPS = [(kh, kw) for kh in range(3) for kw in range(3)]
    # main taps on x_hat, then NCORR correction taps on e_hat
    MM = [(kh, kw, 0) for kh, kw in TAPS] + \
         [(kh, kw, 1) for kh, kw in TAPS[:NCORR]]

    with FastExitTileContext(nc) as tc:
        with tc.tile_pool(name="const", bufs=1) as cpool, \
             tc.tile_pool(name="xin", bufs=4) as xpool, \
             tc.tile_pool(name="outp", bufs=3) as opool, \
             tc.tile_pool(name="eps", bufs=4) as epool, \
             tc.tile_pool(name="psum", bufs=2, space="PSUM") as pspool:

            wts = {kh: cpool.tile([128, 2, 3, COUT], fp8,
                                  name=f"wt{kh}", tag=f"wt{kh}")
                   for kh in range(3)}
            pt = cpool.tile([COUT, 5], fp32)
            b1 = pt[:, 0:1]
            al = pt[:, 1:2]
            pa = pt[:, 2:3]
            b2 = pt[:, 3:4]
            sv = pt[:, 4:5]

            def load_x(b, r0, nrows, ext, tg, halo=None):
                # halo defaults to nrows+2 (rows r0-1 .. r0+nrows); the e_hat
                # tile only needs rows r0-1 .. r0+nrows-2 (kh=0 taps), so its
                # halo is nrows
                halo = nrows + 2 if halo is None else halo
                xb = xpool.tile([128, 2, halo, WP], fp8, tag=tg, name=tg)
                lo = max(r0 - 1, 0)
                hi = min(r0 - 1 + halo, H)
                off = lo - (r0 - 1)
                if off:
                    nc.vector.memset(xb[:, :, 0, :], 0.0)
                if hi - lo + off < halo:
                    nc.vector.memset(xb[:, :, halo - 1, :], 0.0)
                nc.sync.dma_start(
                    xb[:, :, off:off + (hi - lo), :],
                    ext[b, :, :, lo:hi, :])
                return xb

            def emit_block(b, r0, nrows, gsplit, xbs=None):
                if xbs is None:
                    xbs = (load_x(b, r0, nrows, x_ext, "xb"),
                           load_x(b, r0, nrows, e_ext, "eb", halo=nrows))
                xb, eb = xbs

                sb = xpool.tile([COUT, nrows, W], bf16, tag="sb", name="sb")
                nc.sync.dma_start(sb[:], s_ext[b, :, r0:r0 + nrows, :])
                ob = opool.tile([COUT, nrows, W], bf16, tag="ob", name="ob")
                sc = opool.tile([COUT, nrows, W], fp32, tag="sc", name="sc")
                # shortcut = scale * x[:, :128] + b2 (VectorE: ScalarE is the
                # busier engine in the epilogue)
                nc.vector.tensor_scalar(sc[:], sb[:], sv, b2,
                                        OP.mult, OP.add)

                NG = nrows // 4

                def epilogue(g, ps):
                    # prelu(alpha*c + b1) in one ACT op (per-channel slope),
                    # then add the precomputed shortcut
                    r1 = epool.tile([COUT, 512], fp32, tag="r1", name="r1")
                    nc.scalar.activation(r1[:], ps[:], AF.Prelu,
                                         bias=b1, scale=al, alpha=pa)
                    nc.vector.tensor_tensor(
                        ob[:, 4 * g:4 * g + 4, :], r1[:],
                        sc[:, 4 * g:4 * g + 4, :], OP.add)

                for gs in range(0, NG, gsplit):
                    gset = list(range(gs, min(gs + gsplit, NG)))
                    pss = {}
                    for idx, (kh, kw, isc) in enumerate(MM):
                        src = eb if isc else xb
                        for g in gset:
                            if idx == 0:
                                pss[g] = pspool.tile(
                                    [COUT, 512], fp32,
                                    tag=f"ps{g}", name=f"ps{g}")
                            nc.tensor.matmul(
                                pss[g][:],
                                wts[kh][:, :, kw, :],
                                src[:, :, 4 * g + kh:4 * g + kh + 4,
                                    kw:kw + W],
                                start=(idx == 0), stop=(idx == len(MM) - 1),
                                perf_mode=DR)
                    for g in gset:
                        epilogue(g, pss[g])

                nc.sync.dma_start(out_ext[b, :, r0:r0 + nrows, :], ob[:])

            # graduated ramp-up at the start (small DMAs first so PE starts
            # early), steady 16-row blocks, 4-row tail for a short epilogue
            blocks = [(0, 0, 4), (0, 4, 8), (0, 12, 8), (0, 20, 12)]
            r = 32
            while r < H:
                blocks.append((0, r, RB))
                r += RB
            for blk in range(NBLK):
                blocks.append((1, blk * RB, RB))
            blocks[-1] = (B_CORE - 1, (NBLK - 1) * RB, RB - 4)
            blocks.append((B_CORE - 1, H - 4, 4))

            # PE warm-up: dummy matmuls on memset data fill the idle window
            # while the first input DMA lands, releasing the HAM throttle
            dmy = cpool.tile([128, 640], mybir.dt.bfloat16)
            nc.vector.memset(dmy[:, 0:2], 0.0)
            dps = pspool.tile([COUT, 512], fp32, tag="ps0", name="dps")
            for _ in range(20):
                nc.tensor.matmul(dps[:, :256], dmy[:, :128],
                                 dmy[:, 128:384], start=True, stop=True)

            # first block's x before the big const DMAs so PE starts early
            xb_first = load_x(0, 0, 4, x_ext, "xb")
            nc.sync.dma_start(wts[0][:], w_ext[:, :, 0])
            nc.sync.dma_start(wts[1][:], w_ext[:, :, 1])
            eb_first = load_x(0, 0, 4, e_ext, "eb", halo=4)
            nc.sync.dma_start(wts[2][:], w_ext[:, :, 2])
            nc.sync.dma_start(pt[:], p_ext[:])

            for i, (b, r0, nrows) in enumerate(blocks):
                last = i == len(blocks) - 1
                emit_block(b, r0, nrows,
                           min(2, nrows // 4) if last else nrows // 4,
                           xbs=(xb_first, eb_first) if i == 0 else None)

    nc.compile()
    return nc


def kernel(x, conv_w, move1_b, prelu_w, move2_b, scale, _trace=False):
    global LAST_EXEC_NS, LAST_PROFILE
    x = np.asarray(x)
    conv_w = np.asarray(conv_w)
    move1_b = np.asarray(move1_b)
    prelu_w = np.asarray(prelu_w)
    move2_b = np.asarray(move2_b)
    scale = np.asarray(scale)
    assert x.shape == (B_FULL, CIN, H, W), x.shape

    fp8 = ml_dtypes.float8_e4m3

    # --- host-side weight binarization (exact reference math, fp32) ---
    w32 = conv_w.astype(np.float32)
    alpha = np.mean(np.abs(w32), axis=(1, 2, 3))                  # [O]
    # device weights are the signs; alpha is folded into the epilogue
    wl = np.sign(w32).reshape(COUT, 2, 128, 3, 3).transpose(2, 1, 3, 4, 0)
    wl = np.ascontiguousarray(wl).astype(fp8)

    al32 = prelu_w.astype(np.float32)
    b132 = move1_b.astype(np.float32)
    params = np.stack([
        b132,
        alpha,
        al32,
        move2_b.astype(np.float32),
        np.full((COUT,), float(scale[0]), np.float32),
    ], axis=1)
    params = np.ascontiguousarray(params)

    x32 = x.astype(np.float32)
    xq = x32.astype(fp8)                                  # x_hat
    eq = (x32 - xq.astype(np.float32)).astype(fp8)        # e_hat
    # width-pad to WP=130 and lay out [B, 128part, 2chunk, H, WP]
    def _pad(a):
        p = np.zeros((B_FULL, 2, 128, H, WP), fp8)
        p[:, :, :, :, 1:1 + W] = a.reshape(B_FULL, 2, 128, H, W)
        return np.ascontiguousarray(p.transpose(0, 2, 1, 3, 4))
    xq = _pad(xq)
    eq = _pad(eq)
    xs = x32[:, :COUT].astype(ml_dtypes.bfloat16)

    if "nc" not in _cache:
        _cache["nc"] = _build()
    nc = _cache["nc"]

    in_maps = []
    for i in range(N_CORES):
        sl = slice(i * B_CORE, (i + 1) * B_CORE)
        in_maps.append({
            "x8": np.ascontiguousarray(xq[sl]),
            "e8": np.ascontiguousarray(eq[sl]),
            "xs": np.ascontiguousarray(xs[sl]),
            "w8": wl,
            "p": params,
        })

    from concourse.bass_utils import run_bass_kernel_spmd
    res = run_bass_kernel_spmd(nc, in_maps, core_ids=list(range(N_CORES)),
                               trace=_trace)
    LAST_EXEC_NS = res.exec_time_ns
    LAST_PROFILE = res
    out = np.concatenate([res.results[i]["out"] for i in range(N_CORES)],
                         axis=0).astype(np.float32)   # [16,128,128,128]

    # pixel_unshuffle2: [B,C,H,W] -> [B,C*4,H/2,W/2]
    B, C, HH, WW = out.shape
    out = out.reshape(B, C, HH // 2, 2, WW // 2, 2)
    out = out.transpose(0, 1, 3, 5, 2, 4)
    return np.ascontiguousarray(out.reshape(B, C * 4, HH // 2, WW // 2))
